# revision 9
# baseline (speedup 1.0000x reference)
"""CTRNN cell (6 Euler unfolds) on 8 Trainium2 NeuronCores.

Math (per unfold, 6x):
    f     = tanh([x, s] @ W + b)
    s_new = s + 0.1 * (-s + f)  = 0.9*s + 0.1*f

Strategy:
  - Data-parallel over batch: B=8192 -> 1024 rows/core, no cross-core comms.
  - Everything kept TRANSPOSED on-chip (feature dim on SBUF partitions,
    batch on the free dim); host does the cheap numpy transposes.
  - pre = x @ W_top is computed once. Per-unfold matmuls run in *delta*
    form: one PSUM accumulator per output m-tile holds pre + s_k @ W_bot
    across all unfolds, updated with psum += (f_k - s_k) @ (0.1*W_bot).
    That is the 7-logical-matmul FLOP floor.
  - Init matmuls in float32r (fp32 precision, bf16 rate); delta matmuls in
    bf16 (small corrections -> bf16 error is scaled by ~0.1) which also
    enables fast weight loads.
  - PSUM per m-tile is one (128,1024) span (2 banks); matmuls write
    512-wide halves, tanh/DVE read the full row to amortize op overhead.
  - bias is folded into the tanh activation's per-partition bias operand.
  - Input DMAs split across both HWDGE rings (sync + scalar engines);
    f32r rounding casts run on the otherwise-idle scalar engine.
"""

import numpy as np

UNFOLDS = 6
DT = 0.1
B, D, N = 8192, 512, 512
NCORES = 8
BC = B // NCORES          # batch rows per core
CHUNK = 512               # matmul moving-operand free dim (PSUM bank)
NCHUNKS = BC // CHUNK     # 2
P = 128
KT_X = D // P             # k-tiles of W_top
KT_S = N // P             # k-tiles of W_bot
MT = N // P               # m-tiles of the output dim

_compiled_nc = None


def _build_nc():
    import concourse.bass as bass  # noqa: F401
    import concourse.bacc as bacc
    import concourse.tile as tile
    from concourse import mybir

    f32 = mybir.dt.float32
    f32r = mybir.dt.float32r
    bf16 = mybir.dt.bfloat16
    MULT = mybir.AluOpType.mult
    ADD = mybir.AluOpType.add
    TANH = mybir.ActivationFunctionType.Tanh

    nc = bacc.Bacc("TRN2", target_bir_lowering=False, debug=False)

    xT = nc.dram_tensor("xT", [D, BC], f32, kind="ExternalInput").ap()
    sT = nc.dram_tensor("sT", [N, BC], f32, kind="ExternalInput").ap()
    W = nc.dram_tensor("W", [D + N, N], f32, kind="ExternalInput").ap()
    bias = nc.dram_tensor("bias", [N], f32, kind="ExternalInput").ap()
    outT = nc.dram_tensor("outT", [N, BC], f32, kind="ExternalOutput").ap()

    with tile.TileContext(nc) as tc:
        with (
            tc.tile_pool(name="weights", bufs=1) as wpool,
            tc.tile_pool(name="dmain", bufs=3) as dmain,
            tc.tile_pool(name="data", bufs=1) as data,
            tc.tile_pool(name="tmp", bufs=2) as tmpp,
            tc.tile_pool(name="fpool", bufs=3) as fpool,
            tc.tile_pool(name="psum", bufs=1, space="PSUM") as psump,
        ):
            # ---- input DMAs + rounding casts -------------------------------
            # x/s on the sync HWDGE ring; W/bias on the scalar HWDGE ring.
            # Rounding casts to f32r: x on DVE, everything else on the
            # scalar engine (idle at this point).
            wt = []      # W_top rounded f32r, init x-matmuls
            wb01 = []    # 0.1*W_bot f32r, init s-matmul
            wb01h = []   # 0.1*W_bot bf16, delta matmuls
            x_sb = []
            s_sb = []    # persistent state, f32
            s10 = []     # 10*s0 f32r, init-only
            wdma = []
            for j in range(KT_X + KT_S):
                wd = dmain.tile([P, N], f32, tag="wdma", name=f"wdma{j}", bufs=3)
                nc.scalar.dma_start(wd[:], W[j * P:(j + 1) * P, :])
                wdma.append(wd)
            bias_sb = wpool.tile([P, MT], f32, tag="bias", name="bias_sb")
            nc.scalar.dma_start(bias_sb[:], bias.rearrange("(m p) -> p m", p=P))

            for j in range(KT_X):
                t = data.tile([P, BC], f32r, tag=f"x{j}", name=f"x{j}")
                for c in range(NCHUNKS):
                    sl = slice(c * CHUNK, (c + 1) * CHUNK)
                    xd = dmain.tile([P, CHUNK], f32, tag="xdma",
                                    name=f"xdma{j}_{c}", bufs=3)
                    nc.sync.dma_start(xd[:], xT[j * P:(j + 1) * P, sl])
                    nc.vector.tensor_copy(t[:, sl], xd[:])
                x_sb.append(t)
                w = wpool.tile([P, N], f32r, tag=f"wt{j}", name=f"wt{j}")
                nc.scalar.copy(w[:], wdma[j][:])
                wt.append(w)
            for j in range(KT_S):
                t = data.tile([P, BC], f32, tag=f"s{j}", name=f"s{j}")
                nc.gpsimd.dma_start(t[:], sT[j * P:(j + 1) * P, :])
                s_sb.append(t)
                t10 = data.tile([P, BC], f32r, tag=f"s10_{j}", name=f"s10_{j}")
                nc.scalar.mul(t10[:], t[:], 10.0)
                s10.append(t10)
                w = wpool.tile([P, N], f32r, tag=f"wb{j}", name=f"wb01_{j}")
                nc.scalar.mul(w[:], wdma[KT_X + j][:], DT)
                wb01.append(w)
            for j in range(KT_S):
                wh = wpool.tile([P, N], bf16, tag=f"wbh{j}", name=f"wb01h_{j}")
                nc.scalar.mul(wh[:], wdma[KT_X + j][:], DT)
                wb01h.append(wh)

            # ---- persistent PSUM accumulators: pre + s_k @ W_bot ----------
            # one (128, 1024) span per m-tile = 2 banks; matmuls address
            # 512-wide halves, ACT reads the whole span.
            ps = [psump.tile([P, BC], f32, tag=f"ps{m}", name=f"ps{m}")
                  for m in range(MT)]

            def mm_round(weights, rhs_tiles, start, stop, m_outer=False):
                nkt = len(rhs_tiles)
                order = (
                    [(j, m) for m in range(MT) for j in range(nkt)]
                    if m_outer else
                    [(j, m) for j in range(nkt) for m in range(MT)]
                )
                for j, m in order:
                    for c in range(NCHUNKS):
                        nc.tensor.matmul(
                            ps[m][:, c * CHUNK:(c + 1) * CHUNK],
                            lhsT=weights[j][:, m * P:(m + 1) * P],
                            rhs=rhs_tiles[j][:, c * CHUNK:(c + 1) * CHUNK],
                            start=(start and j == 0),
                            stop=(stop and j == nkt - 1),
                            skip_group_check=True,
                        )

            # init: psum = x @ W_top + (10*s0) @ (0.1*W_bot).
            # x part j-outer (starts as soon as x0 lands); s part m-outer so
            # bank 0 finishes its K accumulation early and round 0 overlaps
            # the rest of the init.
            mm_round(wt, x_sb, start=True, stop=False)
            mm_round(wb01, s10, start=False, stop=False, m_outer=True)

            # ---- unfolds ---------------------------------------------------
            for k in range(UNFOLDS):
                last = k == UNFOLDS - 1
                tmp_t = [tmpp.tile([P, BC], bf16, tag=f"tmp{j}",
                                   name=f"tmp{k}_{j}")
                         for j in range(MT)]
                f_t = [fpool.tile([P, BC], f32, tag=f"f{m}", name=f"f{k}_{m}",
                                  bufs=2)
                       for m in range(MT)]
                for m in range(MT):
                    # f = tanh(psum + bias), full (128,1024) span
                    nc.scalar.activation(
                        f_t[m][:], ps[m][:], TANH,
                        bias=bias_sb[:, m:m + 1], scale=1.0,
                    )
                    # tmp = f - s (bf16 out, feeds the delta matmuls)
                    nc.vector.scalar_tensor_tensor(
                        tmp_t[m][:], s_sb[m][:], -1.0, f_t[m][:],
                        op0=MULT, op1=ADD,
                    )
                    if last:
                        # final state + output DMA per m-tile, ASAP
                        nc.vector.scalar_tensor_tensor(
                            s_sb[m][:], tmp_t[m][:], DT, s_sb[m][:],
                            op0=MULT, op1=ADD,
                        )
                        out_eng = nc.sync if m % 2 == 0 else nc.scalar
                        out_eng.dma_start(outT[m * P:(m + 1) * P, :],
                                          s_sb[m][:])
                if not last:
                    # psum += tmp @ (0.1*W_bot)   [bf16]
                    mm_round(wb01h, tmp_t, start=False,
                             stop=(k == UNFOLDS - 2))
                    # s += 0.1 * tmp  (emitted after the matmuls: off the
                    # critical path, fills DVE gaps)
                    for m in range(MT):
                        nc.vector.scalar_tensor_tensor(
                            s_sb[m][:], tmp_t[m][:], DT, s_sb[m][:],
                            op0=MULT, op1=ADD,
                        )

    nc.compile()
    return nc


def _get_nc():
    global _compiled_nc
    if _compiled_nc is None:
        _compiled_nc = _build_nc()
    return _compiled_nc


def kernel(**inputs):
    from concourse.bass_utils import run_bass_kernel_spmd

    x = np.asarray(inputs["inputs"], dtype=np.float32)
    s = np.asarray(inputs["state"], dtype=np.float32)
    W = np.ascontiguousarray(np.asarray(inputs["W"], dtype=np.float32))
    b = np.ascontiguousarray(np.asarray(inputs["bias"], dtype=np.float32))

    xT = np.ascontiguousarray(x.T)  # (D, B)
    sTf = np.ascontiguousarray(s.T)  # (N, B)

    in_maps = []
    for c in range(NCORES):
        sl = slice(c * BC, (c + 1) * BC)
        in_maps.append({
            "xT": np.ascontiguousarray(xT[:, sl]),
            "sT": np.ascontiguousarray(sTf[:, sl]),
            "W": W,
            "bias": b,
        })

    nc = _get_nc()
    res = run_bass_kernel_spmd(nc, in_maps, list(range(NCORES))).results
    outT = np.concatenate([res[c]["outT"] for c in range(NCORES)], axis=1)
    out = np.ascontiguousarray(outT.T).astype(np.float32)
    return (out, out)


# revision 10
# speedup vs baseline: 1.0484x; 1.0484x over previous
"""CTRNN cell (6 Euler unfolds) on 8 Trainium2 NeuronCores.

Math (per unfold, 6x):
    f     = tanh([x, s] @ W + b)
    s_new = s + 0.1 * (-s + f)  = 0.9*s + 0.1*f

Strategy:
  - Data-parallel over batch: B=8192 -> 1024 rows/core, no cross-core comms.
  - Everything kept TRANSPOSED on-chip (feature dim on SBUF partitions,
    batch on the free dim); host does the cheap numpy transposes.
  - pre = x @ W_top is computed once. Per-unfold matmuls run in *delta*
    form: one PSUM accumulator per output m-tile holds pre + s_k @ W_bot
    across all unfolds, updated with psum += (f_k - s_k) @ (0.1*W_bot).
    That is the 7-logical-matmul FLOP floor.
  - Init matmuls in float32r (fp32 precision, bf16 rate); delta matmuls in
    bf16 (small corrections -> bf16 error is scaled by ~0.1) which also
    enables fast weight loads.
  - PSUM per m-tile is one (128,1024) span (2 banks); matmuls write
    512-wide halves, tanh/DVE read the full row to amortize op overhead.
  - bias is folded into the tanh activation's per-partition bias operand.
  - Input DMAs split across both HWDGE rings (sync + scalar engines);
    f32r rounding casts run on the otherwise-idle scalar engine.
"""

import numpy as np

UNFOLDS = 6
DT = 0.1
B, D, N = 8192, 512, 512
NCORES = 8
BC = B // NCORES          # batch rows per core
CHUNK = 512               # matmul moving-operand free dim (PSUM bank)
NCHUNKS = BC // CHUNK     # 2
P = 128
KT_X = D // P             # k-tiles of W_top
KT_S = N // P             # k-tiles of W_bot
MT = N // P               # m-tiles of the output dim

_compiled_nc = None


def _build_nc():
    import concourse.bass as bass  # noqa: F401
    import concourse.bacc as bacc
    import concourse.tile as tile
    from concourse import mybir

    f32 = mybir.dt.float32
    f32r = mybir.dt.float32r
    bf16 = mybir.dt.bfloat16
    MULT = mybir.AluOpType.mult
    ADD = mybir.AluOpType.add
    TANH = mybir.ActivationFunctionType.Tanh

    nc = bacc.Bacc("TRN2", target_bir_lowering=False, debug=False)

    xT = nc.dram_tensor("xT", [D, BC], f32, kind="ExternalInput").ap()
    sT = nc.dram_tensor("sT", [N, BC], f32, kind="ExternalInput").ap()
    W = nc.dram_tensor("W", [D + N, N], f32, kind="ExternalInput").ap()
    bias = nc.dram_tensor("bias", [N], f32, kind="ExternalInput").ap()
    outT = nc.dram_tensor("outT", [N, BC], f32, kind="ExternalOutput").ap()

    with tile.TileContext(nc) as tc:
        with (
            tc.tile_pool(name="weights", bufs=1) as wpool,
            tc.tile_pool(name="dmain", bufs=3) as dmain,
            tc.tile_pool(name="data", bufs=1) as data,
            tc.tile_pool(name="tmp", bufs=2) as tmpp,
            tc.tile_pool(name="fpool", bufs=3) as fpool,
            tc.tile_pool(name="psum", bufs=1, space="PSUM") as psump,
        ):
            # ---- input DMAs + rounding casts -------------------------------
            # x/s on the sync HWDGE ring; W/bias on the scalar HWDGE ring.
            # Rounding casts to f32r: x on DVE, everything else on the
            # scalar engine (idle at this point).
            wt = []      # W_top rounded f32r, init x-matmuls
            wb01 = []    # 0.1*W_bot f32r, init s-matmul
            wb01h = []   # 0.1*W_bot bf16, delta matmuls
            x_sb = []
            s_sb = []    # persistent state, f32
            s10 = []     # 10*s0 f32r, init-only
            wdma = []
            for j in range(KT_X + KT_S):
                wd = dmain.tile([P, N], f32, tag="wdma", name=f"wdma{j}", bufs=3)
                nc.scalar.dma_start(wd[:], W[j * P:(j + 1) * P, :])
                wdma.append(wd)
            bias_sb = wpool.tile([P, MT], f32, tag="bias", name="bias_sb")
            nc.scalar.dma_start(bias_sb[:], bias.rearrange("(m p) -> p m", p=P))

            for j in range(KT_X):
                t = data.tile([P, BC], f32r, tag=f"x{j}", name=f"x{j}")
                for c in range(NCHUNKS):
                    sl = slice(c * CHUNK, (c + 1) * CHUNK)
                    xd = dmain.tile([P, CHUNK], f32, tag="xdma",
                                    name=f"xdma{j}_{c}", bufs=3)
                    nc.sync.dma_start(xd[:], xT[j * P:(j + 1) * P, sl])
                    nc.vector.tensor_copy(t[:, sl], xd[:])
                x_sb.append(t)
                w = wpool.tile([P, N], f32r, tag=f"wt{j}", name=f"wt{j}")
                nc.scalar.copy(w[:], wdma[j][:])
                wt.append(w)
            for j in range(KT_S):
                t = data.tile([P, BC], f32, tag=f"s{j}", name=f"s{j}")
                nc.gpsimd.dma_start(t[:], sT[j * P:(j + 1) * P, :])
                s_sb.append(t)
                t10 = data.tile([P, BC], f32r, tag=f"s10_{j}", name=f"s10_{j}")
                nc.scalar.mul(t10[:], t[:], 10.0)
                s10.append(t10)
                w = wpool.tile([P, N], f32r, tag=f"wb{j}", name=f"wb01_{j}")
                nc.scalar.mul(w[:], wdma[KT_X + j][:], DT)
                wb01.append(w)
            for j in range(KT_S):
                wh = wpool.tile([P, N], bf16, tag=f"wbh{j}", name=f"wb01h_{j}")
                nc.scalar.mul(wh[:], wdma[KT_X + j][:], DT)
                wb01h.append(wh)

            # ---- persistent PSUM accumulators: pre + s_k @ W_bot ----------
            # one (128, 1024) span per m-tile = 2 banks; matmuls address
            # 512-wide halves, ACT reads the whole span.
            ps = [psump.tile([P, BC], f32, tag=f"ps{m}", name=f"ps{m}")
                  for m in range(MT)]

            def mm_round(weights, rhs_tiles, start, stop, m_outer=False):
                nkt = len(rhs_tiles)
                order = (
                    [(j, m) for m in range(MT) for j in range(nkt)]
                    if m_outer else
                    [(j, m) for j in range(nkt) for m in range(MT)]
                )
                for j, m in order:
                    for c in range(NCHUNKS):
                        nc.tensor.matmul(
                            ps[m][:, c * CHUNK:(c + 1) * CHUNK],
                            lhsT=weights[j][:, m * P:(m + 1) * P],
                            rhs=rhs_tiles[j][:, c * CHUNK:(c + 1) * CHUNK],
                            start=(start and j == 0),
                            stop=(stop and j == nkt - 1),
                            skip_group_check=True,
                        )

            # init: psum = x @ W_top + (10*s0) @ (0.1*W_bot).
            # x part j-outer (starts as soon as x0 lands); s part m-outer so
            # bank 0 finishes its K accumulation early and round 0 overlaps
            # the rest of the init.
            mm_round(wt, x_sb, start=True, stop=False)
            mm_round(wb01, s10, start=False, stop=False)

            # ---- unfolds ---------------------------------------------------
            for k in range(UNFOLDS):
                last = k == UNFOLDS - 1
                tmp_t = [tmpp.tile([P, BC], bf16, tag=f"tmp{j}",
                                   name=f"tmp{k}_{j}")
                         for j in range(MT)]
                f_t = [fpool.tile([P, BC], f32, tag=f"f{m}", name=f"f{k}_{m}",
                                  bufs=2)
                       for m in range(MT)]
                for m in range(MT):
                    # f = tanh(psum + bias), full (128,1024) span
                    nc.scalar.activation(
                        f_t[m][:], ps[m][:], TANH,
                        bias=bias_sb[:, m:m + 1], scale=1.0,
                    )
                    # tmp = f - s (bf16 out, feeds the delta matmuls)
                    nc.vector.scalar_tensor_tensor(
                        tmp_t[m][:], s_sb[m][:], -1.0, f_t[m][:],
                        op0=MULT, op1=ADD,
                    )
                    if last:
                        # final state + output DMA per m-tile, ASAP
                        nc.vector.scalar_tensor_tensor(
                            s_sb[m][:], tmp_t[m][:], DT, s_sb[m][:],
                            op0=MULT, op1=ADD,
                        )
                        out_eng = nc.sync if m % 2 == 0 else nc.scalar
                        out_eng.dma_start(outT[m * P:(m + 1) * P, :],
                                          s_sb[m][:])
                if not last:
                    # psum += tmp @ (0.1*W_bot)   [bf16]
                    mm_round(wb01h, tmp_t, start=False,
                             stop=(k == UNFOLDS - 2))
                    # s += 0.1 * tmp  (emitted after the matmuls: off the
                    # critical path, fills DVE gaps)
                    for m in range(MT):
                        nc.vector.scalar_tensor_tensor(
                            s_sb[m][:], tmp_t[m][:], DT, s_sb[m][:],
                            op0=MULT, op1=ADD,
                        )

    nc.compile()
    return nc


def _get_nc():
    global _compiled_nc
    if _compiled_nc is None:
        _compiled_nc = _build_nc()
    return _compiled_nc


def kernel(**inputs):
    from concourse.bass_utils import run_bass_kernel_spmd

    x = np.asarray(inputs["inputs"], dtype=np.float32)
    s = np.asarray(inputs["state"], dtype=np.float32)
    W = np.ascontiguousarray(np.asarray(inputs["W"], dtype=np.float32))
    b = np.ascontiguousarray(np.asarray(inputs["bias"], dtype=np.float32))

    xT = np.ascontiguousarray(x.T)  # (D, B)
    sTf = np.ascontiguousarray(s.T)  # (N, B)

    in_maps = []
    for c in range(NCORES):
        sl = slice(c * BC, (c + 1) * BC)
        in_maps.append({
            "xT": np.ascontiguousarray(xT[:, sl]),
            "sT": np.ascontiguousarray(sTf[:, sl]),
            "W": W,
            "bias": b,
        })

    nc = _get_nc()
    res = run_bass_kernel_spmd(nc, in_maps, list(range(NCORES))).results
    outT = np.concatenate([res[c]["outT"] for c in range(NCORES)], axis=1)
    out = np.ascontiguousarray(outT.T).astype(np.float32)
    return (out, out)
